# revision 1
# baseline (speedup 1.0000x reference)
"""Trainium2 Bass kernel for nn_AddMaskHead (ROI mask head: bilinear pool + concat + conv3x3 + BN + ReLU).

Self-contained: hardcodes shapes B=2, N=256 (512 boxes), C=256, H=96, W=128, P=14.
Shards data-parallel over the 512 boxes across 8 NeuronCores (64 boxes/core; each
core's boxes all come from a single image, so each core only needs its image's
features).
"""

import sys, os, types

sys.path.insert(0, "/opt/trn_rl_repo")

import numpy as np
import concourse.bass as bass
import concourse.mybir as mybir
import concourse.tile as tile
from concourse import bacc
from concourse.masks import make_identity

F32 = mybir.dt.float32
BF16 = mybir.dt.bfloat16
I32 = mybir.dt.int32
ALU = mybir.AluOpType
AF = mybir.ActivationFunctionType

N_CORES = 8
NB = 64            # boxes per core
BATCH = 8          # boxes per inner batch
NBATCH = NB // BATCH
P = 14             # pooler resolution
C = 256            # channels
H, W = 96, 128     # feature map
PQ = P * P         # 196
Q0 = 128           # q-chunk 0 size (q = flattened (y,x) source pixel index)
Q1 = PQ - Q0       # 68


def _axis_static(in_s, out_s=P):
    # mirrors reference._resize_bilinear axis() in exact f32 arithmetic
    s = (np.arange(out_s, dtype=np.float32) + np.float32(0.5)) * np.float32(in_s / out_s) - np.float32(0.5)
    s = np.maximum(s, np.float32(0.0))
    i0 = np.minimum(np.floor(s).astype(np.int32), in_s - 1)
    i1 = np.minimum(i0 + 1, in_s - 1)
    w = (s - i0.astype(np.float32)).astype(np.float32)
    return i0, i1, w


YS0, YS1, WYS = _axis_static(H)
XS0, XS1, WXS = _axis_static(W)


def _consts_p():
    # per-partition constants: [128, 4] = (yv_q0, xv_q0, yv_q1, xv_q1); -1 pads
    arr = np.full((128, 4), -1.0, dtype=np.float32)
    for p in range(128):
        arr[p, 0] = (p // P)
        arr[p, 1] = (p % P)
    for p in range(Q1):
        q = Q0 + p
        arr[p, 2] = (q // P)
        arr[p, 3] = (q % P)
    return arr


def _consts_f():
    # free-dim constants (broadcast to all partitions on device):
    # [0:14] jc = arange(14)+0.5; [14:28] wys; [28:42] 1-wys
    arr = np.zeros((1, 48), dtype=np.float32)
    arr[0, 0:14] = np.arange(P, dtype=np.float32) + np.float32(0.5)
    arr[0, 14:28] = WYS
    arr[0, 28:42] = np.float32(1.0) - WYS
    return arr


def build_kernel():
    nc = bacc.Bacc(None)

    feat = nc.declare_dram_parameter("features", [C, H, W], F32, isOutput=False)
    boxes = nc.declare_dram_parameter("boxes", [NB, 4], F32, isOutput=False)
    mask = nc.declare_dram_parameter("mask", [NB, C, P, P], F32, isOutput=False)
    wt_d = nc.declare_dram_parameter("wt", [128, 4, 9, 256], BF16, isOutput=False)
    epi_d = nc.declare_dram_parameter("epi", [128, 5, 2], F32, isOutput=False)
    cp_d = nc.declare_dram_parameter("consts_p", [128, 4], F32, isOutput=False)
    cf_d = nc.declare_dram_parameter("consts_f", [1, 48], F32, isOutput=False)
    out_d = nc.declare_dram_parameter("out", [NB, C, P, P], F32, isOutput=True)

    mask_v = mask.rearrange("n (ch cp) i j -> cp ch n (i j)", cp=128)
    out_v = out_d.rearrange("n (oh op) i j -> op oh n (i j)", op=128)
    feat_v = feat.rearrange("(ch cp) h w -> cp ch h w", cp=128)

    RC14 = float(np.float32(1.0) / np.float32(P))

    with tile.TileContext(nc) as tc:
        with tc.tile_pool(name="persist", bufs=1) as pp:
            # ---------- persistent tiles ----------
            Wt = pp.tile([128, 4, 9, 256], BF16, tag="Wt")
            # U[s][q, o] = sum_c cf[c, q] * Wc[o, c, s]: folds the ROI pooling
            # into the conv's crops half (contraction over source pixel q)
            Ut = [pp.tile([128, 9, 256], BF16, tag=f"U{qc}", name=f"U{qc}") for qc in range(2)]
            Xb = [pp.tile([128, 2, BATCH, 16, 16], BF16, tag=f"xbuf{i}", name=f"xbuf{i}")
                  for i in range(2)]
            # zero-padded G buffers (double-buffered per q-chunk)
            Gp = [[pp.tile([128, BATCH, 16, 16], BF16, tag=f"gp{qc}{i}", name=f"gp{qc}{i}")
                   for i in range(2)] for qc in range(2)]
            cpt = pp.tile([128, 4], F32, tag="cpt")
            cft = pp.tile([128, 48], F32, tag="cft")
            epi = pp.tile([128, 5, 2], F32, tag="epi")
            scale_e = pp.tile([128, 2], F32, tag="scale_e")
            bias_e = pp.tile([128, 2], F32, tag="bias_e")
            # per-box interpolation data (resident): [128, NB, 14] per axis
            Y0 = pp.tile([128, NB, P], F32, tag="Y0")
            Y1 = pp.tile([128, NB, P], F32, tag="Y1")
            WY = pp.tile([128, NB, P], F32, tag="WY")
            OWY = pp.tile([128, NB, P], F32, tag="OWY")
            X0 = pp.tile([128, NB, P], F32, tag="X0")
            X1 = pp.tile([128, NB, P], F32, tag="X1")
            WX = pp.tile([128, NB, P], F32, tag="WX")
            OWX = pp.tile([128, NB, P], F32, tag="OWX")
            # box-math temps (reused by the two box_math calls)
            bxb = pp.tile([128, NB, 4], F32, tag="bxb")
            abx = pp.tile([128, NB, 4], F32, tag="abx")
            bm_i4 = pp.tile([128, NB, 4], I32, tag="bm_i4")
            bm_f4 = pp.tile([128, NB, 4], F32, tag="bm_f4")
            bm_s = pp.tile([128, NB, P], F32, tag="bm_s")
            bm_f = pp.tile([128, NB, P], F32, tag="bm_f")
            bm_i = pp.tile([128, NB, P], I32, tag="bm_i")
            bm_a = pp.tile([128, NB, P], F32, tag="bm_a")
            bm_d = pp.tile([128, NB], F32, tag="bm_d")
            bm_n = pp.tile([128, NB], F32, tag="bm_n")
            bm_q = pp.tile([128, NB], F32, tag="bm_q")
            bm_h = pp.tile([128, NB], F32, tag="bm_h")

            jc_b = cft[:, 0:14]

            def g_build(tpool, n0, pi, sfx):
                """interpolation matrices G[q, batch, i, j] (bf16, zero-padded
                to 16x16 frames) for one batch, into ping-pong slot pi"""
                Gt = []
                for qc in range(2):
                    shb = [128, BATCH, P]
                    yv = cpt[:, 2 * qc : 2 * qc + 1, None].to_broadcast(shb)
                    xv = cpt[:, 2 * qc + 1 : 2 * qc + 2, None].to_broadcast(shb)
                    my = tpool.tile([128, BATCH, P], F32, tag=f"my{qc}{sfx}", name=f"my{qc}{sfx}")
                    mx = tpool.tile([128, BATCH, P], F32, tag=f"mx{qc}{sfx}", name=f"mx{qc}{sfx}")
                    cmp = tpool.tile([128, BATCH, P], F32, tag=f"cmp{qc}{sfx}", name=f"cmp{qc}{sfx}")
                    bsl = (slice(None), slice(n0, n0 + BATCH), slice(None))
                    nc.vector.tensor_tensor(my[:], Y0[bsl], yv, ALU.is_equal)
                    nc.vector.tensor_mul(my[:], my[:], OWY[bsl])
                    nc.vector.tensor_tensor(cmp[:], Y1[bsl], yv, ALU.is_equal)
                    nc.vector.tensor_mul(cmp[:], cmp[:], WY[bsl])
                    nc.vector.tensor_add(my[:], my[:], cmp[:])
                    nc.vector.tensor_tensor(mx[:], X0[bsl], xv, ALU.is_equal)
                    nc.vector.tensor_mul(mx[:], mx[:], OWX[bsl])
                    nc.vector.tensor_tensor(cmp[:], X1[bsl], xv, ALU.is_equal)
                    nc.vector.tensor_mul(cmp[:], cmp[:], WX[bsl])
                    nc.vector.tensor_add(mx[:], mx[:], cmp[:])
                    G = Gp[qc][pi]
                    shg = [128, BATCH, P, P]
                    nc.vector.tensor_tensor(G[:, :, 1:15, 1:15],
                                            my[:, :, :, None].to_broadcast(shg),
                                            mx[:, :, None, :].to_broadcast(shg), ALU.mult)
                    Gt.append(G)
                return Gt

            def box_math(n0, nn):
                """fill per-axis index/weight arrays for boxes [n0, n0+nn)"""
                ns = slice(n0, n0 + nn)
                t, fr, ti = abx[:, ns], bm_f4[:, ns], bm_i4[:, ns]
                nc.vector.tensor_scalar_mul(t[:], bxb[:, ns], 0.125)
                nc.vector.tensor_copy(ti[:], t[:])
                nc.vector.tensor_copy(fr[:], ti[:])
                nc.vector.tensor_tensor(ti[:].bitcast(F32), fr[:], t[:], ALU.is_gt)
                nc.vector.tensor_sub(t[:], fr[:], ti[:].bitcast(F32))
                d, nlt, beq, adj = bm_d[:, ns], bm_n[:, ns], bm_q[:, ns], bm_h[:, ns]
                for ax in range(2):  # 0: x (cols 0,2), 1: y (cols 1,3)
                    a_io, b_io = t[:, :, ax], t[:, :, 2 + ax]
                    nc.vector.tensor_sub(d[:], b_io, a_io)
                    nc.vector.tensor_scalar(nlt[:], d[:], 1.0, None, ALU.is_lt)
                    nc.vector.tensor_scalar(beq[:], b_io, float(P), None, ALU.is_equal)
                    nc.vector.tensor_mul(adj[:], nlt[:], beq[:])
                    nc.vector.tensor_sub(a_io, a_io, adj[:])
                    nc.vector.tensor_add(b_io, b_io, nlt[:])
                    nc.vector.tensor_sub(b_io, b_io, adj[:])
                nwid, him1 = bm_d[:, ns], bm_n[:, ns]
                s, frs, si, i0c = bm_s[:, ns], bm_f[:, ns], bm_i[:, ns], bm_a[:, ns]
                sh3 = [128, nn, P]
                for ax, (I0, I1, Wf, OWf) in enumerate(
                    [(X0, X1, WX, OWX), (Y0, Y1, WY, OWY)]
                ):
                    lo_b = t[:, :, ax][:, :, None].to_broadcast(sh3)
                    nc.vector.tensor_sub(nwid[:], t[:, :, 2 + ax], t[:, :, ax])
                    nc.vector.tensor_scalar_sub(him1[:], nwid[:], 1.0)
                    h_b = him1[:, :, None].to_broadcast(sh3)
                    nc.vector.tensor_tensor(s[:], nwid[:, :, None].to_broadcast(sh3),
                                            jc_b[:, None, :].to_broadcast(sh3), ALU.mult)
                    nc.vector.tensor_scalar(s[:], s[:], RC14, -0.5, ALU.mult, ALU.add)
                    nc.vector.tensor_scalar(s[:], s[:], 0.0, None, ALU.max)
                    nc.vector.tensor_copy(si[:], s[:])
                    nc.vector.tensor_copy(frs[:], si[:])
                    nc.vector.tensor_tensor(si[:].bitcast(F32), frs[:], s[:], ALU.is_gt)
                    nc.vector.tensor_sub(i0c[:], frs[:], si[:].bitcast(F32))
                    nc.vector.tensor_tensor(i0c[:], i0c[:], h_b, ALU.min)
                    nc.vector.tensor_sub(Wf[:, ns], s[:], i0c[:])
                    nc.vector.tensor_scalar(OWf[:, ns], Wf[:, ns], -1.0, 1.0, ALU.mult, ALU.add)
                    nc.vector.tensor_add(I0[:, ns], i0c[:], lo_b)
                    nc.vector.tensor_scalar_add(i0c[:], i0c[:], 1.0)
                    nc.vector.tensor_tensor(i0c[:], i0c[:], h_b, ALU.min)
                    nc.vector.tensor_add(I1[:, ns], i0c[:], lo_b)

            # ---------- phase 0 ----------
            with tc.tile_pool(name="ph0", bufs=1) as p0, \
                 tc.tile_pool(name="ps0", bufs=1, space="PSUM") as ps0:

                # --- tiny DMAs first on SP (bx1 gates box math), then weights
                ones1 = p0.tile([1, 128], F32, tag="ones1")
                nc.gpsimd.memset(ones1[:], 1.0)
                bx1 = p0.tile([1, NB * 4], F32, tag="bx1")
                nc.sync.dma_start(bx1[:], boxes.rearrange("n f -> (n f)")[None, :])
                cf1 = p0.tile([1, 48], F32, tag="cf1")
                nc.sync.dma_start(cf1[:], cf_d[:])
                nc.sync.dma_start(cpt[:], cp_d[:])

                # --- weights: bf16, host-laid-out -> straight into Wt (no cast);
                #     first-needed chunks first, ci2/ci3 after the feature rows
                nc.sync.dma_start(Wt[:, 0:1].rearrange("p a b c -> p (a b c)"),
                                  wt_d[:, 0:1].rearrange("p a b c -> p (a b c)"))

                # --- mask batch 0 ch0 prefetch (gates the first conv matmuls)
                mst0 = p0.tile([128, 2, BATCH, PQ], F32, tag="mst0")
                nc.sync.dma_start(mst0[:, 0], mask_v[:, 0, 0:BATCH])
                nc.sync.dma_start(Wt[:, 1:2].rearrange("p a b c -> p (a b c)"),
                                  wt_d[:, 1:2].rearrange("p a b c -> p (a b c)"))

                # --- broadcasts via K=1 matmul with ones (PE is idle here)
                psb = ps0.tile([128, 256], F32, tag="psb")
                nc.tensor.matmul(psb[:], ones1[:], bx1[:])
                nc.scalar.copy(bxb[:].rearrange("p n f -> p (n f)"), psb[:])
                psf = ps0.tile([128, 48], F32, tag="psf")
                nc.tensor.matmul(psf[:], ones1[:], cf1[:])
                nc.scalar.copy(cft[:], psf[:])

                # --- feature rows: YS1[i] == YS0[i]+1 always, so load row pairs.
                #     YS0 is piecewise-affine (stride-7 runs) -> few strided DMAs
                assert (YS1 == YS0 + 1).all()
                runs = []  # (i_start, count, step)
                rs = 0
                for i in range(1, P + 1):
                    if i == P or (i - rs >= 2 and YS0[i] - YS0[i - 1] != YS0[rs + 1] - YS0[rs]):
                        step = int(YS0[rs + 1] - YS0[rs]) if i - rs >= 2 else 1
                        runs.append((rs, i - rs, step))
                        rs = i
                R01 = p0.tile([128, 2, P, 2, W], F32, tag="R01")
                for ch in range(2):
                    for (i0r, cnt, step) in runs:
                        base = int(YS0[i0r])
                        for r in range(2):  # r = 0: YS0 rows, r = 1: YS1 rows
                            nc.sync.dma_start(
                                R01[:, ch, i0r : i0r + cnt, r],
                                feat_v[:, ch, base + r : base + r + (cnt - 1) * step + 1 : step])

                # --- mask batch 0 ch1, then late weight chunks
                nc.sync.dma_start(mst0[:, 1], mask_v[:, 1, 0:BATCH])
                nc.sync.dma_start(Wt[:, 2:4].rearrange("p a b c -> p (a b c)"),
                                  wt_d[:, 2:4].rearrange("p a b c -> p (a b c)"))

                # --- X padding margins (interiors are rewritten every batch)
                for i in range(2):
                    Xv = Xb[i]
                    nc.vector.memset(Xv[:, :, :, 0, :], 0.0)
                    nc.vector.memset(Xv[:, :, :, 15, :], 0.0)
                    nc.vector.memset(Xv[:, :, :, 1:15, 0], 0.0)
                    nc.vector.memset(Xv[:, :, :, 1:15, 15], 0.0)

                # --- mask b0 casts on the scalar engine (DVE is busy)
                mst0_v = mst0[:].rearrange("p ch n (i j) -> p ch n i j", j=P)
                nc.scalar.copy(Xb[0][:, 0, :, 1:15, 1:15], mst0_v[:, 0])
                nc.scalar.copy(Xb[0][:, 1, :, 1:15, 1:15], mst0_v[:, 1])

                # --- G frame padding + U pad rows (one-time zeroing, gpsimd)
                for qc in range(2):
                    nc.gpsimd.memset(Gp[qc][0][:], 0.0)
                    nc.gpsimd.memset(Gp[qc][1][:], 0.0)
                nc.gpsimd.memset(Ut[1][:], 0.0)

                # --- box math + G for batch 0 only (unblocks crops b0 early)
                box_math(0, BATCH)
                Gt0 = g_build(p0, 0, 0, "b0")

                # --- concat-features (cf): x-lerp on narrow row pairs, then y-lerp
                cfx = p0.tile([128, 2, P, 2, P], F32, tag="cfx")  # (ch, i, r, j)
                tmpx = p0.tile([128, 2, P, 2], F32, tag="tmpx")
                for j in range(P):
                    nc.vector.tensor_scalar_mul(cfx[:, :, :, :, j], R01[:, :, :, :, int(XS0[j])],
                                                float(np.float32(1.0) - WXS[j]))
                    nc.vector.tensor_scalar_mul(tmpx[:], R01[:, :, :, :, int(XS1[j])], float(WXS[j]))
                    nc.vector.tensor_add(cfx[:, :, :, :, j], cfx[:, :, :, :, j], tmpx[:])
                cfv = pp.tile([128, 2, P, P], F32, tag="cfv")
                tmpy = p0.tile([128, 2, P, P], F32, tag="tmpy")
                shc = [128, 2, P, P]
                nc.vector.tensor_tensor(cfv[:], cfx[:, :, :, 0, :],
                                        cft[:, None, 28:42, None].to_broadcast(shc), ALU.mult)
                nc.vector.tensor_tensor(tmpy[:], cfx[:, :, :, 1, :],
                                        cft[:, None, 14:28, None].to_broadcast(shc), ALU.mult)
                nc.vector.tensor_add(cfv[:], cfv[:], tmpy[:])
                cfb = pp.tile([128, 2, P, P], BF16, tag="cfb")
                nc.vector.tensor_copy(cfb[:], cfv[:])

                # --- epilogue scalars
                nc.sync.dma_start(epi[:].rearrange("p a b -> p (a b)"),
                                  epi_d.rearrange("p a b -> p (a b)"))
                tmp_e = p0.tile([128, 2], F32, tag="tmp_e")
                eps_t = p0.tile([128, 1], F32, tag="eps_t")
                nc.vector.memset(eps_t[:], 1e-5)
                nc.scalar.activation(tmp_e[:], epi[:, 4, :], AF.Sqrt, bias=eps_t[:], scale=1.0)
                nc.vector.reciprocal(scale_e[:], tmp_e[:])
                nc.vector.tensor_mul(scale_e[:], scale_e[:], epi[:, 1, :])
                nc.vector.tensor_sub(bias_e[:], epi[:, 0, :], epi[:, 3, :])
                nc.vector.tensor_mul(bias_e[:], bias_e[:], scale_e[:])
                nc.vector.tensor_add(bias_e[:], bias_e[:], epi[:, 2, :])

            # ---------- main loop ----------
            with tc.tile_pool(name="loop", bufs=2) as lp, \
                 tc.tile_pool(name="gpool", bufs=2) as gp, \
                 tc.tile_pool(name="psc", bufs=2, space="PSUM") as psc, \
                 tc.tile_pool(name="psv", bufs=6, space="PSUM") as psv:

                for b in range(NBATCH):
                    n0 = b * BATCH
                    X = Xb[b % 2]

                    # --- mask features into padded X interior (b0 prefetched in phase 0)
                    if b > 0:
                        mst = lp.tile([128, 2, BATCH, PQ], F32, tag="mst")
                        for ch in range(2):
                            nc.sync.dma_start(mst[:, ch], mask_v[:, ch, n0 : n0 + BATCH])
                        mst_v = mst[:].rearrange("p ch n (i j) -> p ch n i j", j=P)
                        for ch in range(2):
                            nc.vector.tensor_copy(X[:, ch, :, 1:15, 1:15], mst_v[:, ch])

                    # --- G matrices: [128(q), BATCH, 16, 16] padded bf16 per q-chunk
                    Gt = Gt0 if b == 0 else g_build(gp, n0, b % 2, "")

                    # --- conv: mask half (ci 0,1) of o-chunk 0 first -> PE busy
                    #     while this batch's pooling operands are produced
                    ost = lp.tile([128, 2, BATCH, PQ], F32, tag="ost")
                    pcv0 = [psv.tile([128, 2 * PQ], F32, tag="conv", name=f"cnv_{b}_0_{pr}")
                            for pr in range(4)]

                    def conv_mask(pcv, oc):
                        for ci in range(2):
                            for sdy in range(3):
                                for sdx in range(3):
                                    first = (ci == 0 and sdy == 0 and sdx == 0)
                                    lhsT = Wt[:, ci, 3 * sdy + sdx, oc * 128 : oc * 128 + 128]
                                    for pr in range(4):
                                        rhs = X[:, ci, 2 * pr : 2 * pr + 2,
                                                sdy : sdy + P, sdx : sdx + P]
                                        nc.tensor.matmul(pcv[pr][:], lhsT, rhs,
                                                         start=first, stop=False)

                    def conv_crops(pcv, oc):
                        # pooled-crops half, pooling folded in: contract over q
                        # with U stationary and zero-padded G moving
                        for qc in range(2):
                            for sdy in range(3):
                                for sdx in range(3):
                                    last = (qc == 1 and sdy == 2 and sdx == 2)
                                    lhsT = Ut[qc][:, 3 * sdy + sdx, oc * 128 : oc * 128 + 128]
                                    for pr in range(4):
                                        rhs = Gt[qc][:, 2 * pr : 2 * pr + 2,
                                                     sdy : sdy + P, sdx : sdx + P]
                                        nc.tensor.matmul(pcv[pr][:], lhsT, rhs,
                                                         start=False, stop=last)

                    conv_mask(pcv0, 0)

                    if b == 0:
                        # build U[s][q, o] = sum_c cf[c, q] Wc[o, c, s]; emitted
                        # after the first conv MMs so PE doesn't stall on cfv
                        cfv_f = cfb[:].rearrange("p c i j -> p c (i j)")
                        for qc in range(2):
                            qn = 128 if qc == 0 else Q1
                            qs = slice(qc * 128, qc * 128 + qn)
                            for s in range(9):
                                psU = psc.tile([128, 256], F32, tag="upsum",
                                               name=f"ups{qc}_{s}")
                                for cc in range(2):
                                    nc.tensor.matmul(psU[:qn], cfv_f[:, cc, qs],
                                                     Wt[:, 2 + cc, s, :],
                                                     start=(cc == 0), stop=(cc == 1))
                                nc.scalar.copy(Ut[qc][:qn, s, :], psU[:qn])

                    # box math for the NEXT batch (overlaps this batch's conv)
                    if b + 1 < NBATCH:
                        box_math(n0 + BATCH, BATCH)

                    # --- conv: pooled-crops half of o-chunk 0, then o-chunk 1
                    conv_crops(pcv0, 0)
                    for pr in range(4):
                        nc.scalar.activation(
                            ost[:, 0, 2 * pr : 2 * pr + 2],
                            pcv0[pr][:].rearrange("p (n q) -> p n q", n=2),
                            AF.Relu, bias=bias_e[:, 0:1], scale=scale_e[:, 0:1],
                        )
                    pcv1 = [psv.tile([128, 2 * PQ], F32, tag="conv", name=f"cnv_{b}_1_{pr}")
                            for pr in range(4)]
                    conv_mask(pcv1, 1)
                    conv_crops(pcv1, 1)
                    nc.sync.dma_start(out_v[:, 0, n0 : n0 + BATCH], ost[:, 0])
                    for pr in range(4):
                        nc.scalar.activation(
                            ost[:, 1, 2 * pr : 2 * pr + 2],
                            pcv1[pr][:].rearrange("p (n q) -> p n q", n=2),
                            AF.Relu, bias=bias_e[:, 1:2], scale=scale_e[:, 1:2],
                        )
                        if pr == 1:
                            nc.sync.dma_start(out_v[:, 1, n0 : n0 + 4], ost[:, 1, 0:4])
                    nc.sync.dma_start(out_v[:, 1, n0 + 4 : n0 + BATCH], ost[:, 1, 4:BATCH])

    nc.compile()
    return nc


# ---------------------------------------------------------------------------
# host-side sharding / unsharding
# ---------------------------------------------------------------------------

def _prep_in_maps(features, proposal_boxes, mask_features, conv_w, conv_b,
                  bn_gamma, bn_beta, bn_mean, bn_var):
    features = np.asarray(features, dtype=np.float32)
    proposal_boxes = np.asarray(proposal_boxes, dtype=np.float32)
    mask_features = np.asarray(mask_features, dtype=np.float32)
    conv_w = np.asarray(conv_w, dtype=np.float32)
    # weight layout: [cout=256, cin=512, 3, 3] -> [cin_par=128, cin_hi=4, 9, cout=256], bf16
    import ml_dtypes
    wt = np.ascontiguousarray(
        conv_w.reshape(256, 4, 128, 3 * 3).transpose(2, 1, 3, 0)
    ).astype(ml_dtypes.bfloat16)
    epi = np.stack([np.asarray(x, dtype=np.float32) for x in
                    (conv_b, bn_gamma, bn_beta, bn_mean, bn_var)])  # [5, 256]
    epi = np.ascontiguousarray(epi.reshape(5, 2, 128).transpose(2, 0, 1)).astype(np.float32)
    cp = _consts_p()
    cfc = _consts_f()

    in_maps = []
    for i in range(N_CORES):
        img = i // (N_CORES // 2)
        n0 = (i * NB) % 256
        in_maps.append({
            "features": np.ascontiguousarray(features[img]),
            "boxes": np.ascontiguousarray(proposal_boxes[img, n0 : n0 + NB]),
            "mask": np.ascontiguousarray(mask_features[i * NB : (i + 1) * NB]),
            "wt": wt,
            "epi": epi,
            "consts_p": cp,
            "consts_f": cfc,
        })
    return in_maps


_NC_CACHE = {}


def _get_nc():
    if "nc" not in _NC_CACHE:
        _NC_CACHE["nc"] = build_kernel()
    return _NC_CACHE["nc"]


def _install_ntff_shim():
    """antenv.axon_hooks is missing in this image; shim it so trace=True works."""
    try:
        import antenv
        if hasattr(antenv, "axon_hooks"):
            return
        from trn_agent_boot.trn_boot import _ntff_profile_via_ctypes
        mod = types.ModuleType("antenv.axon_hooks")
        _h = [None]
        mod.set_axon_ntff_profile_hook = lambda h: _h.__setitem__(0, h)
        mod.get_axon_ntff_profile_hook = lambda: _h[0]
        sys.modules["antenv.axon_hooks"] = mod
        antenv.axon_hooks = mod
        mod.set_axon_ntff_profile_hook(_ntff_profile_via_ctypes("/opt/axon/libaxon_pjrt.so"))
    except Exception:
        pass


def run(trace=False, tmpdir=None, **inputs):
    from concourse.bass_utils import run_bass_kernel_spmd

    if trace:
        _install_ntff_shim()
    nc = _get_nc()
    in_maps = _prep_in_maps(**inputs)
    res = run_bass_kernel_spmd(nc, in_maps, core_ids=list(range(N_CORES)),
                               trace=trace, tmpdir=tmpdir)
    out = np.concatenate([np.asarray(res.results[i]["out"]) for i in range(N_CORES)], axis=0)
    return out.astype(np.float32), res


def kernel(**inputs):
    out, _ = run(trace=False, **inputs)
    return out

